# revision 11
# baseline (speedup 1.0000x reference)
"""Trainium2 Bass kernel for nn_LowRankDynamicConv.

Math (per sample b):
  combined = [phrase_slot[b] | eos]                       [N, 2C]
  h        = relu(combined @ W1 + b1)                     [N, 4C]
  proj     = (h @ W2 + b2) viewed as [N*C, R]             [4096, 32]
  y        = x[b] @ proj   with x[b] = context_emb[b] as  [T, N*C]
  out[t]   = relu(LN(sum_{k,j} y[t + j - pad_k] @ M_kj + bo))
  where M_kj[r, co] = sum_d kjoin[kj, r, d] * Wo[k_block*C + d, co]

Key perf structure (v2):
  - x ships as fp8 e3m4 (scaled x2 on host; W2/b2 pre-halved to compensate),
    streamed straight into the stage-3 matmul as the moving operand against
    bf16 proj weights (mixed-dtype matmul).  Halves the biggest HBM stream.
  - All heavy DMAs use host-prepared layouts with per-partition-contiguous
    8-16KB runs (128 descriptors per DMA) - HWDGE descriptor generation at
    ~2.4ns/desc was the old bottleneck (1024x2KB descs per DMA).
  - The two skinny (M=32) matmul stages use 4-way PE column tiling
    (tile_position=(0,32j)): four concurrent M=32 matmuls in separate
    32-column groups of the PE array, each draining to its own PSUM bank
    partition slice.
  - W2 streams strictly before x (all 8 chunk DMAs dep-free up front) so
    proj is ready the moment the x stream begins; stage 3 chases x chunks.
  - y lives in a [128, 516] SBUF tile: partition group 32*(2b+hf) holds
    sample b's T-half hf, so every PSUM evacuation is lane-aligned; the
    conv-tap shifted copies (yk tiles) are small SBUF->SBUF DMAs.
  - LayerNorm pipeline is spread across ACT (normalize via per-partition
    scale/bias), DVE (stats, relu) and GpSimd (gamma/beta) so no single
    engine paces the emit tail.
"""
import sys

sys.path.insert(0, "/opt/trn_rl_repo")

import ml_dtypes
import numpy as np

import concourse.bass as bass  # noqa: F401  (bass types used via bacc)
import concourse.mybir as mybir
import concourse.tile as tile
from concourse import bacc
from concourse.bass_utils import run_bass_kernel_spmd
from concourse.masks import make_identity

F32 = mybir.dt.float32
BF16 = mybir.dt.bfloat16
FP8 = mybir.dt.float8e3
RELU = mybir.ActivationFunctionType.Relu
SQRT = mybir.ActivationFunctionType.Sqrt
IDENT = mybir.ActivationFunctionType.Identity
BF = ml_dtypes.bfloat16
E3 = ml_dtypes.float8_e3m4

X_FP8 = True               # ship x as fp8 e3m4 (x2, W2/b2 pre-halved)

NCORES = 8
BPC = 2                    # samples per core
T, N, C, R = 1024, 16, 256, 32
NCF = N * C                # 4096 flattened (n, c) contraction dim
CH = NCF // 128            # 32 nc-chunks of 128
GCH = 8                    # nc-chunks per x DMA tile
XT = CH // GCH             # x DMA tiles per sample (4)
QT = 256                   # T-quarter (stage-3 col-group free dim)
YCW = T + 4                # consolidated y width incl 2+2 pad columns
# (wo-block, temporal offset) per fused tap, in k1 | k3 | k5 order
JOFF = [(0, 0), (1, -1), (1, 0), (1, 1), (2, -2), (2, -1), (2, 0), (2, 1), (2, 2)]


def _broadcast_ap(ap, parts):
    """DMA access pattern replicating a 1D/2D DRAM tensor across `parts` partitions."""
    a = ap
    return bass.AP(tensor=a.tensor, offset=a.offset, ap=[[0, parts]] + list(a.ap))


def _build():
    nc = bacc.Bacc("TRN2", num_devices=NCORES)

    xq = nc.dram_tensor("xq", [BPC, 128, CH, T], FP8 if X_FP8 else BF16,
                        kind="ExternalInput")
    phrase = nc.dram_tensor("phrase", [BPC * N, C], BF16, kind="ExternalInput")
    eos = nc.dram_tensor("eos", [C], BF16, kind="ExternalInput")
    w1h = nc.dram_tensor("w1h", [128, 4, 4 * C], BF16, kind="ExternalInput")
    b1 = nc.dram_tensor("b1", [4 * C], F32, kind="ExternalInput")
    w2h = nc.dram_tensor("w2h", [128, 8, 8, 1024], BF16, kind="ExternalInput")
    b2 = nc.dram_tensor("b2", [C * R], F32, kind="ExternalInput")
    kjh = nc.dram_tensor("kjh", [R, 9, C], BF16, kind="ExternalInput")
    woh = nc.dram_tensor("woh", [128, 6, C], BF16, kind="ExternalInput")
    lnp = nc.dram_tensor("lnp", [3, C], F32, kind="ExternalInput")
    out = nc.dram_tensor("out", [BPC, 2, 128, 4 * C], BF16,
                         kind="ExternalOutput")

    with tile.TileContext(nc) as tc:
        with tc.tile_pool(name="keep", bufs=1) as keep, \
             tc.tile_pool(name="pXg", bufs=4) as pXg, \
             tc.tile_pool(name="dram", bufs=1, space="DRAM") as dram:
            ident = keep.tile([128, 128], BF16)
            make_identity(nc, ident)

            # LN params + output bias (gamma|beta|bo), one broadcast DMA
            # (small loads ride the scalar HWDGE ring so the big W2 stream on
            # the sync ring starts immediately)
            lnsb = keep.tile([128, 3, C], F32)
            nc.scalar.dma_start(lnsb, _broadcast_ap(lnp[:, :], 128))
            gsb = lnsb[:, 0, :]
            bsb = lnsb[:, 1, :]
            bosb = lnsb[:, 2, :]
            epsb = keep.tile([128, 1], F32)
            nc.vector.memset(epsb, 1e-5)
            # b2 regrouped to the post-reshard proj layout: [c%128, c-half, r]
            b2v = keep.tile([128, 2, R], F32)
            nc.scalar.dma_start(b2v, b2[:].rearrange("(c2 p r) -> p c2 r", p=128, r=R))
            # bf16 gamma/beta for the elementwise post-normalize ops
            gb16 = keep.tile([128, 1, C], BF16)
            nc.vector.tensor_copy(gb16[:, 0, :], gsb)
            bb16 = keep.tile([128, 1, C], BF16)
            nc.vector.tensor_copy(bb16[:, 0, :], bsb)

            # stage-3 lhsT tiles [c%128 part, (b, c-half, n), r] bf16
            projf = keep.tile([128, BPC * CH, R], BF16)
            # fused conv+output weights M_kj [r part, tap, co], plus stacked
            # rhs tiles for the 3-matmul emit: taps 0-3 | taps 4-7 | tap 8+bo
            msb = keep.tile([R, 9, C], BF16)
            mst0 = keep.tile([128, C], BF16)
            mst1 = keep.tile([128, C], BF16)
            mcat = keep.tile([R + 1, C], BF16)
            # h^T persists through phase A
            hT = keep.tile([128, 8, BPC * N], BF16)

            # ---- phase A -------------------------------------------------------
            with tc.tile_pool(name="pA", bufs=1) as pA, \
                 tc.tile_pool(name="pW2", bufs=8) as pW2, \
                 tc.tile_pool(name="pAs", bufs=2) as pAs, \
                 tc.tile_pool(name="psA", bufs=2, space="PSUM") as psA, \
                 tc.tile_pool(name="psW", bufs=4, space="PSUM") as psW:
                # small loads on the scalar ring; W2 chunks own the sync ring
                phsb = pA.tile([BPC * N, C], BF16)
                nc.scalar.dma_start(phsb, phrase[:, :])
                eossb = pA.tile([128, 2], BF16)
                nc.scalar.dma_start(eossb, eos[:].rearrange("(o p) -> p o", p=128))
                w1sb = pA.tile([128, 4, 4 * C], BF16)
                nc.scalar.dma_start(w1sb, w1h[:, :, :])
                b1sb = pA.tile([128, 8], F32)
                nc.scalar.dma_start(b1sb, b1[:].rearrange("(mo p) -> p mo", p=128))
                kjf = pA.tile([R, 9, C], BF16)
                nc.scalar.dma_start(kjf, kjh[:, :, :])
                wof = pA.tile([128, 6, C], BF16)
                nc.scalar.dma_start(wof, woh[:, :, :])

                w2sb = []
                for j8 in range(8):
                    w = pW2.tile([128, 8, 1024], BF16, tag="w2", name=f"w2_{j8}")
                    nc.sync.dma_start(w, w2h[:, j8, :, :])
                    w2sb.append(w)

                # combined^T [c2%128 part, ko, bn] bf16
                combT = pA.tile([128, 4, BPC * N], BF16)
                for ko in range(2):
                    pt = psA.tile([128, BPC * N], BF16, tag="t")
                    nc.tensor.transpose(pt, phsb[:, ko * 128:(ko + 1) * 128],
                                        ident[:BPC * N, :BPC * N])
                    nc.vector.tensor_copy(combT[:, ko, :], pt)
                for o in range(2):
                    nc.vector.tensor_copy(
                        combT[:, 2 + o, :],
                        eossb[:, o:o + 1].to_broadcast((128, BPC * N)))

                # h^T [m%128 part, mo, bn] = relu(W1^T combined + b1), bf16
                for mo in range(8):
                    ph = psA.tile([128, BPC * N], F32, tag="t")
                    for ko in range(4):
                        nc.tensor.matmul(ph, w1sb[:, ko, mo * 128:(mo + 1) * 128],
                                         combT[:, ko, :],
                                         start=(ko == 0), stop=(ko == 3))
                    nc.scalar.activation(out=hT[:, mo, :], in_=ph, func=RELU,
                                         bias=b1sb[:, mo:mo + 1], scale=1.0)

                # M_kj = kjoin_kj @ Wo_block: transpose kjoin taps, then contract
                kjT = pA.tile([128, 2 * 9, R], BF16)
                for ji in range(9):
                    for dc in range(2):
                        pt = psA.tile([128, R], BF16, tag="t")
                        nc.tensor.transpose(pt, kjf[:, ji, dc * 128:(dc + 1) * 128],
                                            ident[:R, :R])
                        nc.vector.tensor_copy(kjT[:, ji * 2 + dc, :], pt)
                for ji, (kb, _off) in enumerate(JOFF):
                    pm = psA.tile([R, C], F32, tag="t")
                    for dc in range(2):
                        nc.tensor.matmul(pm, kjT[:, ji * 2 + dc, :],
                                         wof[:, kb * 2 + dc, :],
                                         start=(dc == 0), stop=(dc == 1))
                    nc.vector.tensor_copy(msb[:, ji, :], pm)
                # stack the emit rhs: taps 0-3 / 4-7 on 128 partitions; tap 8
                # plus the bo row on 33 (gpsimd queue: tiny, off the big FIFO)
                for q in range(4):
                    nc.gpsimd.dma_start(mst0[q * R:(q + 1) * R, :], msb[:, q, :])
                    nc.gpsimd.dma_start(mst1[q * R:(q + 1) * R, :], msb[:, 4 + q, :])
                nc.gpsimd.dma_start(mcat[0:R, :], msb[:, 8, :])
                nc.vector.tensor_copy(mcat[R:R + 1, :], bosb[0:1, :])

                # proj rows via 4-way column-tiled matmuls: set s covers
                # m-cols [s*2048, (s+1)*2048); group j streams its own 512
                # W2 columns into PE column group j concurrently (M=32 each)
                scratch = dram.tile([BPC * N, C * R], BF16)

                def reshard(c2):
                    # scratch cols [c2*4096, +4096) are final once sets
                    # 2*c2, 2*c2+1 have written; reshard them immediately so
                    # stage 3 is not gated on the whole of phase A
                    for b in range(BPC):
                        dst = projf[:, b * CH + c2 * N:b * CH + c2 * N + N, :]
                        nc.gpsimd.dma_start(
                            dst,
                            scratch[b * N:(b + 1) * N, c2 * 4096:(c2 + 1) * 4096]
                            .rearrange("n (p r) -> p n r", p=128, r=R))
                        nc.vector.tensor_add(
                            dst, dst,
                            b2v[:, c2:c2 + 1, :].to_broadcast((128, N, R)))

                for s in range(4):
                    psum = [psW.tile([128, 512], F32, tag="pj", name=f"pj{s}_{j}")
                            for j in range(4)]
                    for ko in range(8):
                        for j in range(4):
                            wch = w2sb[2 * s + j // 2]
                            q2 = j % 2
                            nc.tensor.matmul(
                                psum[j][32 * j:32 * j + 32, :],
                                hT[:, ko, :],
                                wch[:, ko, q2 * 512:(q2 + 1) * 512],
                                start=(ko == 0), stop=(ko == 7),
                                tile_position=(0, 32 * j))
                    pjsb = pAs.tile([128, 512], BF16, tag="pjsb")
                    for j in range(4):
                        nc.vector.tensor_copy(pjsb[32 * j:32 * j + 32, :],
                                              psum[j][32 * j:32 * j + 32, :])
                        nc.scalar.dma_start(
                            scratch[:, s * 2048 + j * 512:s * 2048 + (j + 1) * 512],
                            pjsb[32 * j:32 * j + 32, :])
                    if s == 1:
                        reshard(0)
                    elif s == 3:
                        reshard(1)

            # ---- phase X: streamed x tiles, col-tiled stage 3, emit -----------
            xgs = {}
            for b in range(BPC):
                for g in range(XT):
                    xg = pXg.tile([128, GCH, T], FP8 if X_FP8 else BF16,
                                  tag="xg", name=f"xg{b}_{g}")
                    nc.sync.dma_start(xg, xq[b, :, g * GCH:(g + 1) * GCH, :])
                    xgs[(b, g)] = xg

            with tc.tile_pool(name="pXw", bufs=6) as pXw, \
                 tc.tile_pool(name="pY", bufs=2) as pY, \
                 tc.tile_pool(name="obuf4", bufs=4) as obuf4, \
                 tc.tile_pool(name="yp", bufs=4, space="PSUM") as yp, \
                 tc.tile_pool(name="op", bufs=4, space="PSUM") as op:

                def s3(b, pys):
                    # stage 3: the four T-quarters of sample b accumulate
                    # across the 32 nc-chunks in four concurrent PE column
                    # groups (M=32 each), one PSUM bank per quarter
                    for ch in range(CH):
                        lhs = projf[:, b * CH + (ch % 2) * N + ch // 2, :]
                        xg = xgs[(b, ch // GCH)]
                        for q in range(4):
                            nc.tensor.matmul(
                                pys[q][32 * q:32 * q + 32, :],
                                lhs,
                                xg[:, ch % GCH, q * QT:(q + 1) * QT],
                                start=(ch == 0), stop=(ch == CH - 1),
                                tile_position=(0, 32 * q))

                def yfin(b, pys):
                    # evacuate quarters (lane-aligned), consolidate into the
                    # contiguous padded ysbc (+ ones row for the bo matmul),
                    # then one shifted copy per conv tap
                    ysbq = pY.tile([128, QT], BF16, tag="ysbq", name=f"ysbq{b}")
                    for q in range(4):
                        nc.vector.tensor_copy(ysbq[32 * q:32 * q + 32, :],
                                              pys[q][32 * q:32 * q + 32, :])
                    ysbc = pY.tile([R + 1, YCW], BF16, tag="ysbc", name=f"ysbc{b}")
                    nc.vector.memset(ysbc[0:R, 0:2], 0.0)
                    nc.vector.memset(ysbc[0:R, YCW - 2:YCW], 0.0)
                    nc.gpsimd.memset(ysbc[R:R + 1, :], 1.0)
                    for q in range(4):
                        eng = nc.scalar if q % 2 == 0 else nc.gpsimd
                        eng.dma_start(ysbc[0:R, 2 + q * QT:2 + (q + 1) * QT],
                                      ysbq[32 * q:32 * q + 32, :])
                    yk0 = pY.tile([128, T], BF16, tag="yk0", name=f"yk0_{b}")
                    yk1 = pY.tile([128, T], BF16, tag="yk1", name=f"yk1_{b}")
                    for q, (_kb, off) in enumerate(JOFF[0:4]):
                        nc.scalar.dma_start(yk0[q * R:(q + 1) * R, :],
                                            ysbc[0:R, off + 2:off + 2 + T])
                    for q, (_kb, off) in enumerate(JOFF[4:8]):
                        nc.gpsimd.dma_start(yk1[q * R:(q + 1) * R, :],
                                            ysbc[0:R, off + 2:off + 2 + T])
                    return ysbc, yk0, yk1

                def emit(b, ysbc, yk0, yk1):
                    # emit: 3 stacked matmuls (tap 8 + bo ride directly on the
                    # ysbc rows incl. its ones row) + LN + relu per tile; the
                    # gamma/beta/relu passes run pair-merged on two tiles
                    zn2 = None
                    for ts in range(T // 128):
                        po = op.tile([128, C], F32, tag="o")
                        nc.tensor.matmul(po, yk0[:, ts * 128:(ts + 1) * 128],
                                         mst0, start=True, stop=False)
                        nc.tensor.matmul(po, yk1[:, ts * 128:(ts + 1) * 128],
                                         mst1, start=False, stop=False)
                        nc.tensor.matmul(po, ysbc[:, 4 + ts * 128:4 + (ts + 1) * 128],
                                         mcat, start=False, stop=True)
                        st = pXw.tile([128, 6], F32, tag="st")
                        nc.vector.bn_stats(out=st, in_=po)
                        mv = pXw.tile([128, 2], F32, tag="mv")
                        nc.vector.bn_aggr(out=mv, in_=st)
                        rs = pXw.tile([128, 1], F32, tag="rs")
                        nc.scalar.activation(out=rs, in_=mv[:, 1:2], func=SQRT,
                                             bias=epsb, scale=1.0)
                        nc.vector.reciprocal(rs, rs)
                        nmr = pXw.tile([128, 1], F32, tag="nmr")
                        nc.vector.tensor_scalar(nmr, mv[:, 0:1], rs[:, 0:1], -1.0,
                                                mybir.AluOpType.mult,
                                                mybir.AluOpType.mult)
                        if ts % 2 == 0:
                            zn2 = pXw.tile([128, 2, C], BF16, tag="zn2")
                        nc.scalar.activation(out=zn2[:, ts % 2, :], in_=po,
                                             func=IDENT,
                                             bias=nmr[:, 0:1], scale=rs[:, 0:1])
                        if ts % 4 == 0:
                            ob = obuf4.tile([128, 4, C], BF16, tag="ob4",
                                            name=f"ob{b}_{ts}")
                        if ts % 2 == 1:
                            zg2 = pXw.tile([128, 2, C], BF16, tag="zg2")
                            nc.gpsimd.tensor_mul(zg2, zn2,
                                                 gb16[:, 0:1, :].to_broadcast((128, 2, C)))
                            nc.gpsimd.tensor_add(zg2, zg2,
                                                 bb16[:, 0:1, :].to_broadcast((128, 2, C)))
                            h = (ts % 4) - 1
                            nc.vector.tensor_scalar_max(ob[:, h:h + 2, :], zg2, 0.0)
                        if ts % 4 == 3:
                            nc.scalar.dma_start(
                                out[b, ts // 4, :, :]
                                .rearrange("p (q c) -> p q c", q=4), ob)

                for b in range(BPC):
                    pys = [yp.tile([128, QT], F32, tag="y", name=f"py{b}_{q}")
                           for q in range(4)]
                    s3(b, pys)
                    ysbc, yk0, yk1 = yfin(b, pys)
                    emit(b, ysbc, yk0, yk1)

    nc.compile()
    return nc


_NC = None


def _get_nc():
    global _NC
    if _NC is None:
        _NC = _build()
    return _NC


def _shard(inputs):
    """Split full inputs into per-core input maps (layout/cast only)."""
    x = np.asarray(inputs["context_emb"], dtype=np.float32)
    B = x.shape[0]
    assert B == NCORES * BPC
    # [B, T, N, C] -> [B, NCF, T] -> [B, 128, CH, T], fp8 e3m4 at 2x scale
    xT = np.swapaxes(x.reshape(B, T, NCF), 1, 2)
    xT = np.ascontiguousarray(
        np.swapaxes(xT.reshape(B, CH, 128, T), 1, 2))
    if X_FP8:
        xq = np.clip(xT * 2.0, -15.0, 15.0).astype(E3)
        w2scale = 0.5
    else:
        xq = xT.astype(BF)
        w2scale = 1.0
    ph = np.asarray(inputs["phrase_slot"], dtype=np.float32)
    w2 = np.asarray(inputs["W2"], dtype=np.float32) * w2scale
    w2h = np.ascontiguousarray(
        w2.reshape(8, 128, 8, 1024).transpose(1, 2, 0, 3)).astype(BF)
    w1 = np.asarray(inputs["W1"], dtype=np.float32)
    w1h = np.ascontiguousarray(
        w1.reshape(4, 128, 4 * C).transpose(1, 0, 2)).astype(BF)
    kjoin = np.concatenate(
        [np.moveaxis(inputs[f"k{k}"], 2, 0) for k in (1, 3, 5)],
        axis=0)  # [9, 32, 256]
    kjh = np.ascontiguousarray(np.moveaxis(kjoin, 1, 0)).astype(BF)  # [r, j, d]
    wo = np.asarray(inputs["Wo"], dtype=np.float32)
    woh = np.ascontiguousarray(
        wo.reshape(6, 128, C).transpose(1, 0, 2)).astype(BF)
    shared = {
        "eos": np.asarray(inputs["eos_slot"], dtype=np.float32).reshape(C).astype(BF),
        "w1h": w1h,
        "b1": np.ascontiguousarray(inputs["b1"], dtype=np.float32),
        "w2h": w2h,
        "b2": np.ascontiguousarray(
            np.asarray(inputs["b2"], dtype=np.float32) * w2scale),
        "kjh": kjh,
        "woh": woh,
        "lnp": np.ascontiguousarray(np.stack([
            np.asarray(inputs["gamma"], dtype=np.float32),
            np.asarray(inputs["beta"], dtype=np.float32),
            np.asarray(inputs["bo"], dtype=np.float32)])),
    }
    in_maps = []
    for i in range(NCORES):
        m = dict(shared)
        m["xq"] = np.ascontiguousarray(xq[i * BPC:(i + 1) * BPC])
        m["phrase"] = np.ascontiguousarray(
            ph[i * BPC:(i + 1) * BPC].reshape(BPC * N, C).astype(BF))
        in_maps.append(m)
    return in_maps


def _run(inputs, **kwargs):
    nc = _get_nc()
    res = run_bass_kernel_spmd(nc, _shard(inputs), core_ids=list(range(NCORES)),
                               **kwargs)
    outs = [r["out"] for r in res.results]
    full = np.concatenate(outs, axis=0).reshape(NCORES * BPC, 2, 128, 4, C)
    # [b, s, p, q, c] -> t = (s*4 + q)*128 + p
    full = np.ascontiguousarray(full.transpose(0, 1, 3, 2, 4)).reshape(
        NCORES * BPC, T, C)
    return full.astype(np.float32), res


def kernel(**inputs) -> np.ndarray:
    out, _ = _run(inputs)
    return out
